# revision 1
# baseline (speedup 1.0000x reference)
"""LoRA-MLP kernel for 8x TRN2 NeuronCores (SPMD data-parallel over batch).

Math (per batch b):
    z1 = (x @ v) / IN            [F, R]
    z  = (z1 @ u.T) / R          [F, OUT]
    y  = gelu(x @ W.T + fc_bias + z + b)

Device formulation (per core, 4 batches), all PSUM-accumulated per f-tile:
    psum[f, o] = ones[1,f].T @ bias[1,o]          (K=1: fc_bias + b)
               + sum_k xT[k][:, f].T @ WT[k][:, o]  (8 K-tiles of 128)
               + z1T[:, f].T @ uT[:, o]             (K=16 LoRA)
    out = gelu(psum)   (ScalarE, PSUM -> SBUF fp32)
    z1T[r, f] = sum_k vs[k].T @ xT[k]  on PE, copied PSUM->SBUF via ScalarE.

All matmul operands bf16 (host-cast/laid out); fp32 accumulation in PSUM.
Sync-wait budget note: this codegen allows roughly one semaphore wait per
compute instruction (2 for DMA), so pools are sized for zero slot reuse and
each producer/consumer pair crosses engines exactly once.
"""

import sys

for _p in ("/opt/trn_rl_repo", "/opt/pypackages"):
    if _p not in sys.path:
        sys.path.append(_p)

import numpy as np
import ml_dtypes

B, F, IN, OUT, R = 32, 512, 1024, 1024, 16
NCORES = 8
BPC = B // NCORES  # batches per core = 4
KT = IN // 128  # 8 K-tiles
FT = F // 128  # 4 F-tiles per batch
BF16 = ml_dtypes.bfloat16

_COMPILED = {}


def _build_nc():
    import concourse.tile as tile
    from concourse import bacc, mybir

    # Bacc (not raw Bass): its compile() runs generate_event_semaphores,
    # which splits multi-sem waits — walrus codegen allows only one sync
    # wait per instruction.
    nc = bacc.Bacc(None)
    bf = mybir.dt.bfloat16
    f32 = mybir.dt.float32

    xt = nc.declare_dram_parameter("xt", [BPC, 128, KT, F], bf, isOutput=False)
    wt = nc.declare_dram_parameter("wt", [128, KT, OUT], bf, isOutput=False)
    vs = nc.declare_dram_parameter("vs", [BPC, 128, KT, R], bf, isOutput=False)
    ut = nc.declare_dram_parameter("ut", [BPC, R, OUT], bf, isOutput=False)
    bias = nc.declare_dram_parameter("bias", [BPC, 1, OUT], bf, isOutput=False)
    ones = nc.declare_dram_parameter("ones", [1, 128], bf, isOutput=False)
    y = nc.declare_dram_parameter("y", [BPC, FT, 128, OUT], f32, isOutput=True)

    GELU = mybir.ActivationFunctionType.Gelu

    with tile.TileContext(nc) as tc:
        with (
            tc.tile_pool(name="const", bufs=1) as const_pool,
            tc.tile_pool(name="xin", bufs=BPC) as xin_pool,
            tc.tile_pool(name="small", bufs=BPC) as small_pool,
            tc.tile_pool(name="out", bufs=FT * BPC) as out_pool,
            tc.tile_pool(name="psum", bufs=6, space="PSUM") as psum_pool,
            tc.tile_pool(name="zpsum", bufs=2, space="PSUM") as zpsum_pool,
        ):
            wt_sb = const_pool.tile([128, KT, OUT], bf)
            nc.sync.dma_start(out=wt_sb[:], in_=wt[:])
            ones_sb = const_pool.tile([1, 128], bf)
            nc.sync.dma_start(out=ones_sb[:], in_=ones[:])

            z1_tiles = [
                const_pool.tile([R, F], bf, name=f"z1_{i}", tag=f"z1_{i}")
                for i in range(BPC)
            ]

            for b in range(BPC):
                xt_sb = xin_pool.tile([128, KT, F], bf, tag="xt")
                nc.sync.dma_start(out=xt_sb[:], in_=xt[b])
                vs_sb = small_pool.tile([128, KT, R], bf, tag="vs")
                nc.sync.dma_start(out=vs_sb[:], in_=vs[b])
                ut_sb = small_pool.tile([R, OUT], bf, tag="ut")
                nc.sync.dma_start(out=ut_sb[:], in_=ut[b])
                bias_sb = small_pool.tile([1, OUT], bf, tag="bias")
                nc.sync.dma_start(out=bias_sb[:], in_=bias[b])

                # Stage 1: z1T[r, f] = sum_k vs[k].T @ xT[k]  -> [16, F] PSUM
                z1_ps = zpsum_pool.tile([R, F], f32, tag="z1ps")
                for k in range(KT):
                    nc.tensor.matmul(
                        z1_ps[:],
                        lhsT=vs_sb[:, k, :],
                        rhs=xt_sb[:, k, :],
                        start=(k == 0),
                        stop=(k == KT - 1),
                    )
                z1_sb = z1_tiles[b]
                nc.scalar.copy(z1_sb[:], z1_ps[:])

                # Stage 2: bias + main matmul + LoRA, accumulated in PSUM.
                for ft in range(FT):
                    fsl = slice(ft * 128, (ft + 1) * 128)
                    ps0 = psum_pool.tile([128, 512], f32, tag="ps")
                    ps1 = psum_pool.tile([128, 512], f32, tag="ps")
                    nc.tensor.matmul(
                        ps0[:], lhsT=ones_sb[:], rhs=bias_sb[:, 0:512],
                        start=True, stop=False,
                    )
                    nc.tensor.matmul(
                        ps1[:], lhsT=ones_sb[:], rhs=bias_sb[:, 512:1024],
                        start=True, stop=False,
                    )
                    for k in range(KT):
                        lhsT = xt_sb[:, k, fsl]
                        nc.tensor.matmul(
                            ps0[:], lhsT=lhsT, rhs=wt_sb[:, k, 0:512],
                            start=False, stop=False,
                        )
                        nc.tensor.matmul(
                            ps1[:], lhsT=lhsT, rhs=wt_sb[:, k, 512:1024],
                            start=False, stop=False,
                        )
                    nc.tensor.matmul(
                        ps0[:], lhsT=z1_sb[:, fsl], rhs=ut_sb[:, 0:512],
                        start=False, stop=True,
                    )
                    nc.tensor.matmul(
                        ps1[:], lhsT=z1_sb[:, fsl], rhs=ut_sb[:, 512:1024],
                        start=False, stop=True,
                    )
                    # One [128, 1024] tile per f-tile: both gelu halves land in
                    # it, then a single 512KB store (4KB/partition lines).
                    # Bacc's generate_event_semaphores legalizes the DMA's two
                    # ACT waits.
                    o01 = out_pool.tile([128, OUT], f32, tag="o")
                    nc.scalar.activation(o01[:, 0:512], ps0[:], GELU)
                    nc.scalar.activation(o01[:, 512:1024], ps1[:], GELU)
                    nc.sync.dma_start(out=y[b, ft], in_=o01[:])
    nc.finalize()
    return nc


def _shard_inputs(x, u, v, b, W, fc_bias):
    """Build per-core device input dicts (host-side layout + bf16 cast)."""
    # xt[c][bb, p, k, f] = x[4c+bb, f, 128k+p]
    xt = np.ascontiguousarray(
        x.reshape(B, F, KT, 128).transpose(0, 3, 2, 1)
    ).astype(BF16)
    # wt[p, k, o] = W[o, 128k+p]
    wt = np.ascontiguousarray(W.reshape(OUT, KT, 128).transpose(2, 1, 0)).astype(BF16)
    # vs[bb, p, k, r] = v[bb, 0, 128k+p, r] / (IN*R)
    vs = np.ascontiguousarray(
        (v[:, 0] / float(IN * R)).reshape(B, KT, 128, R).transpose(0, 2, 1, 3)
    ).astype(BF16)
    # ut[bb, r, o] = u[bb, 0, o, r]
    ut = np.ascontiguousarray(u[:, 0].transpose(0, 2, 1)).astype(BF16)
    bias = (fc_bias[None, None, :] + b).astype(BF16)  # [B, 1, OUT]

    in_maps = []
    for c in range(NCORES):
        s = slice(c * BPC, (c + 1) * BPC)
        in_maps.append(
            {
                "xt": xt[s],
                "wt": wt,
                "vs": vs[s],
                "ut": ut[s],
                "bias": np.ascontiguousarray(bias[s]),
                "ones": np.ones((1, 128), dtype=BF16),
            }
        )
    return in_maps


def _run(in_maps, trace=False, **kw):
    from concourse import bass_utils

    key = "nc"
    if key not in _COMPILED:
        _COMPILED[key] = _build_nc()
    nc = _COMPILED[key]
    res = bass_utils.run_bass_kernel_spmd(
        nc, in_maps, list(range(NCORES)), trace=trace, **kw
    )
    return res


def kernel(x, u, v, b, W, fc_bias):
    x = np.asarray(x, dtype=np.float32)
    u = np.asarray(u, dtype=np.float32)
    v = np.asarray(v, dtype=np.float32)
    b = np.asarray(b, dtype=np.float32)
    W = np.asarray(W, dtype=np.float32)
    fc_bias = np.asarray(fc_bias, dtype=np.float32)

    in_maps = _shard_inputs(x, u, v, b, W, fc_bias)
    res = _run(in_maps, trace=False)
    outs = [r["y"].reshape(BPC, F, OUT) for r in res.results]
    return np.concatenate(outs, axis=0).astype(np.float32)



# revision 2
# speedup vs baseline: 25069.8912x; 25069.8912x over previous
"""LoRA-MLP kernel for 8x TRN2 NeuronCores (SPMD data-parallel over batch).

Math (per batch b):
    z1 = (x @ v) / IN            [F, R]
    z  = (z1 @ u.T) / R          [F, OUT]
    y  = gelu(x @ W.T + fc_bias + z + b)

Device formulation (per core, 4 batches), output-channel-stationary:
    z1T[r, f]  = sum_k vs[k].T @ xT[k]          (PE, K=8x128; PSUM [16, F])
    psum[o, f] = sum_k wT[k][:, o].T @ xT[k][:, f]   (8 K-tiles of 128)
               + uT[:, o].T @ z1T                     (K=16 LoRA)
    yT[o, f]   = gelu(psum + biasvec[o])   (ScalarE per-partition bias, -> bf16)

With o on the PSUM partition dim, fc_bias + b is a per-partition scalar, so
the ScalarE activation applies it for free (no K=1 bias matmuls), and the
store is bf16 (half the HBM/store traffic).  Host un-transposes yT.

All matmul operands bf16 (host-cast/laid out); fp32 accumulation in PSUM.
`reps` repeats the whole per-core pass inside one NEFF (same output each
time) so a bench harness can measure steady-state per-pass HW time by the
slope between two reps values.
"""

import sys

for _p in ("/opt/trn_rl_repo", "/opt/pypackages"):
    if _p not in sys.path:
        sys.path.append(_p)

import numpy as np
import ml_dtypes

B, F, IN, OUT, R = 32, 512, 1024, 1024, 16
NCORES = 8
BPC = B // NCORES  # batches per core = 4
KT = IN // 128  # 8 K-tiles
OT = OUT // 128  # 8 output-channel tiles
BF16 = ml_dtypes.bfloat16

_COMPILED = {}


def _build_nc(reps=1):
    import concourse.tile as tile
    from concourse import bacc, mybir

    # Bacc (not raw Bass): its compile() runs generate_event_semaphores,
    # which splits multi-sem waits — walrus codegen allows only one sync
    # wait per instruction.
    nc = bacc.Bacc(None)
    bf = mybir.dt.bfloat16
    f32 = mybir.dt.float32

    xt = nc.declare_dram_parameter("xt", [BPC, 128, KT, F], bf, isOutput=False)
    wt = nc.declare_dram_parameter("wt", [128, KT, OUT], bf, isOutput=False)
    vs = nc.declare_dram_parameter("vs", [BPC, 128, KT, R], bf, isOutput=False)
    ut = nc.declare_dram_parameter("ut", [BPC, R, OUT], bf, isOutput=False)
    biasv = nc.declare_dram_parameter("biasv", [128, BPC * OT], f32, isOutput=False)
    y = nc.declare_dram_parameter("y", [BPC, OT, 128, F], bf, isOutput=True)

    GELU = mybir.ActivationFunctionType.Gelu

    with tile.TileContext(nc) as tc:
        with (
            tc.tile_pool(name="const", bufs=1) as const_pool,
            tc.tile_pool(name="xin", bufs=BPC) as xin_pool,
            tc.tile_pool(name="small", bufs=BPC) as small_pool,
            tc.tile_pool(name="zsb", bufs=2) as zsb_pool,
            tc.tile_pool(name="out", bufs=8) as out_pool,
            tc.tile_pool(name="psum", bufs=6, space="PSUM") as psum_pool,
            tc.tile_pool(name="zpsum", bufs=2, space="PSUM") as zpsum_pool,
        ):
            wt_sb = const_pool.tile([128, KT, OUT], bf)
            nc.sync.dma_start(out=wt_sb[:], in_=wt[:])
            bias_sb = const_pool.tile([128, BPC * OT], f32)
            nc.sync.dma_start(out=bias_sb[:], in_=biasv[:])

            for _ in range(reps):
                for b in range(BPC):
                    xt_sb = xin_pool.tile([128, KT, F], bf, tag="xt")
                    nc.sync.dma_start(out=xt_sb[:], in_=xt[b])
                    vs_sb = small_pool.tile([128, KT, R], bf, tag="vs")
                    nc.sync.dma_start(out=vs_sb[:], in_=vs[b])
                    ut_sb = small_pool.tile([R, OUT], bf, tag="ut")
                    nc.sync.dma_start(out=ut_sb[:], in_=ut[b])

                    # Stage 1: z1T[r, f] = sum_k vs[k].T @ xT[k] -> [16, F] PSUM
                    z1_ps = zpsum_pool.tile([R, F], f32, tag="z1ps")
                    for k in range(KT):
                        nc.tensor.matmul(
                            z1_ps[:],
                            lhsT=vs_sb[:, k, :],
                            rhs=xt_sb[:, k, :],
                            start=(k == 0),
                            stop=(k == KT - 1),
                        )
                    z1_sb = zsb_pool.tile([R, F], bf, tag="z1")
                    nc.vector.tensor_copy(out=z1_sb[:], in_=z1_ps[:])

                    # Stage 2: per o-tile, main matmul + LoRA in PSUM, then
                    # fused bias+gelu on ScalarE straight to bf16.
                    for ot in range(OT):
                        osl = slice(ot * 128, (ot + 1) * 128)
                        ps = psum_pool.tile([128, F], f32, tag="ps")
                        for k in range(KT):
                            nc.tensor.matmul(
                                ps[:],
                                lhsT=wt_sb[:, k, osl],
                                rhs=xt_sb[:, k, :],
                                start=(k == 0),
                                stop=False,
                            )
                        nc.tensor.matmul(
                            ps[:], lhsT=ut_sb[:, osl], rhs=z1_sb[:],
                            start=False, stop=True,
                        )
                        o_sb = out_pool.tile([128, F], bf, tag="o")
                        bidx = b * OT + ot
                        nc.scalar.activation(
                            o_sb[:], ps[:], GELU,
                            bias=bias_sb[:, bidx : bidx + 1],
                        )
                        nc.sync.dma_start(out=y[b, ot], in_=o_sb[:])
    nc.finalize()
    return nc


def _shard_inputs(x, u, v, b, W, fc_bias):
    """Build per-core device input dicts (host-side layout + bf16 cast)."""
    # xt[c][bb, p, k, f] = x[4c+bb, f, 128k+p]
    xt = np.ascontiguousarray(
        x.reshape(B, F, KT, 128).transpose(0, 3, 2, 1)
    ).astype(BF16)
    # wt[p, k, o] = W[o, 128k+p]
    wt = np.ascontiguousarray(W.reshape(OUT, KT, 128).transpose(2, 1, 0)).astype(BF16)
    # vs[bb, p, k, r] = v[bb, 0, 128k+p, r] / (IN*R)
    vs = np.ascontiguousarray(
        (v[:, 0] / float(IN * R)).reshape(B, KT, 128, R).transpose(0, 2, 1, 3)
    ).astype(BF16)
    # ut[bb, r, o] = u[bb, 0, o, r]
    ut = np.ascontiguousarray(u[:, 0].transpose(0, 2, 1)).astype(BF16)
    # biasv[c][p, bb*OT+ot] = fc_bias[128*ot+p] + b[4c+bb, 0, 128*ot+p]
    bias_full = (fc_bias[None, :] + b[:, 0]).astype(np.float32)  # [B, OUT]
    biasv = np.ascontiguousarray(
        bias_full.reshape(B, OT, 128).transpose(2, 0, 1)
    )  # [128, B, OT]

    in_maps = []
    for c in range(NCORES):
        s = slice(c * BPC, (c + 1) * BPC)
        in_maps.append(
            {
                "xt": xt[s],
                "wt": wt,
                "vs": vs[s],
                "ut": ut[s],
                "biasv": np.ascontiguousarray(biasv[:, s, :]).reshape(128, BPC * OT),
            }
        )
    return in_maps


def _run(in_maps, trace=False, reps=1, **kw):
    from concourse import bass_utils

    key = reps
    if key not in _COMPILED:
        _COMPILED[key] = _build_nc(reps)
    nc = _COMPILED[key]
    res = bass_utils.run_bass_kernel_spmd(
        nc, in_maps, list(range(NCORES)), trace=trace, **kw
    )
    return res


def kernel(x, u, v, b, W, fc_bias):
    x = np.asarray(x, dtype=np.float32)
    u = np.asarray(u, dtype=np.float32)
    v = np.asarray(v, dtype=np.float32)
    b = np.asarray(b, dtype=np.float32)
    W = np.asarray(W, dtype=np.float32)
    fc_bias = np.asarray(fc_bias, dtype=np.float32)

    in_maps = _shard_inputs(x, u, v, b, W, fc_bias)
    res = _run(in_maps, trace=False)
    outs = []
    for r in res.results:
        yt = np.asarray(r["y"], dtype=np.float32)  # [BPC, OT, 128, F]
        outs.append(yt.transpose(0, 3, 1, 2).reshape(BPC, F, OUT))
    return np.concatenate(outs, axis=0)
